# revision 1
# baseline (speedup 1.0000x reference)
"""Trainium2 Bass kernel for multi-head causal self-attention.

Tensor-parallel over 8 NeuronCores: each core owns 2 of the 16 heads.
Per core (SPMD, identical program, different weight shards):
  - QKV projections for its 2 heads (contract over d_model, X^T streamed)
  - causal attention for its 2 heads (scores kept transposed [k, q];
    softmax denominator via a ones-column fused into the PV matmul)
  - output projection partial (its 256 rows of Wo), inlined per q-block
Host: shards weights, pre-transposes X, sums the 8 partials, adds bo.

All matmuls run as float32r (fp32 data, FP22 multiply) with moving dim
>= 256 so the tensor engine streams at 1 row/cycle.
"""
import numpy as np
from contextlib import ExitStack

import concourse.bass as bass
import concourse.tile as tile
from concourse import bacc, mybir
from concourse.bass_utils import run_bass_kernel_spmd

# Problem shape (hardcoded per contract)
B, S, D = 2, 2048, 2048
H, DH = 16, 128
N_CORES = 8
HL = H // N_CORES          # heads per core = 2
DHL = HL * DH              # 256
SC = 256                   # s-chunk for projections
NSC = S // SC              # 8 chunks per batch
NKB = S // 128             # 16 key blocks per batch
NQI = S // 512             # 4 q-chunks of 512 per batch
NDC = D // 128             # 16 contraction blocks

F32 = mybir.dt.float32
F32R = mybir.dt.float32r
AF = mybir.ActivationFunctionType

_cached_nc = None


def _mm(nc, out, lhsT, rhs, start, stop):
    nc.tensor.matmul(out, lhsT, rhs, start=start, stop=stop)


def build_nc(dbg=False):
    nc = bacc.Bacc("TRN2", target_bir_lowering=False, debug=False, num_devices=N_CORES)

    xt = nc.dram_tensor("xt", [B, D, S], F32R, kind="ExternalInput").ap()
    wq = nc.dram_tensor("wq", [D, DHL], F32R, kind="ExternalInput").ap()
    wk = nc.dram_tensor("wk", [D, DHL], F32R, kind="ExternalInput").ap()
    wv = nc.dram_tensor("wv", [D, DHL], F32R, kind="ExternalInput").ap()
    bqt_d = nc.dram_tensor("bqt", [128, HL], F32, kind="ExternalInput").ap()
    wo = nc.dram_tensor("wo", [HL, 128, D], F32R, kind="ExternalInput").ap()
    mask_d = nc.dram_tensor("maskt", [128, 128], F32R, kind="ExternalInput").ap()
    vones_d = nc.dram_tensor("vones", [128, 1, 1], F32R, kind="ExternalInput").ap()
    ident_d = nc.dram_tensor("ident", [128, 128], F32R, kind="ExternalInput").ap()
    out = nc.dram_tensor("out", [B, S, D], F32, kind="ExternalOutput").ap()

    if dbg:
        qt_dump = nc.dram_tensor("qt_dump", [B, 128, HL, S], F32,
                                 kind="ExternalOutput").ap()
        kt_dump = nc.dram_tensor("kt_dump", [B, 128, HL, S], F32,
                                 kind="ExternalOutput").ap()
        vcat_dump = nc.dram_tensor("vcat_dump", [B, 128, NKB, 257], F32,
                                   kind="ExternalOutput").ap()
        an_dump = nc.dram_tensor("an_dump", [B, HL, S, 128], F32,
                                 kind="ExternalOutput").ap()
        rc_dump = nc.dram_tensor("rc_dump", [B, HL, S // 128, 128, 1], F32,
                                 kind="ExternalOutput").ap()

    wq_r = wq.rearrange("(n p) d -> p n d", p=128)
    wk_r = wk.rearrange("(n p) d -> p n d", p=128)
    wv_r = wv.rearrange("(n p) d -> p n d", p=128)

    with tile.TileContext(nc) as tc, ExitStack() as ctx:
        pp = ctx.enter_context(tc.tile_pool(name="persist", bufs=1))

        # allocate persistent tiles; DMAs are emitted in dependency-critical
        # order below (wq + first x chunk gate the first matmuls)
        wq_t = pp.tile([128, NDC, DHL], F32R)
        wk_t = pp.tile([128, NDC, DHL], F32R)
        wv_t = pp.tile([128, NDC, DHL], F32R)
        wo_t = pp.tile([128, HL, D], F32R)
        bqt = pp.tile([128, HL], F32)
        mask = pp.tile([128, 128], F32R)
        ident = pp.tile([128, 128], F32R)

        nc.sync.dma_start(out=wq_t, in_=wq_r)
        nc.sync.dma_start(out=bqt, in_=bqt_d)

        xp = ctx.enter_context(tc.tile_pool(name="xtp", bufs=2))

        for b in range(B):
            with tc.tile_pool(name=f"bat{b}", bufs=1) as bp:
                qt = bp.tile([128, HL, S], F32R)   # Q^T per head
                kt = bp.tile([128, HL, S], F32R)   # K^T per head
                # [V_h0 | 1 | V_h1] per key block
                vcat = bp.tile([128, NKB, 257], F32R)
                nc.sync.dma_start(out=vcat[:, :, 128:129],
                                  in_=vones_d.to_broadcast([128, NKB, 1]))

                xt_r = xt[b].rearrange("(n p) s -> p n s", p=128)

                # ---- QKV projections ----
                with tc.tile_pool(name=f"psp{b}", bufs=1, space="PSUM") as psp:
                    for sc in range(NSC):
                        xt_t = xp.tile([128, NDC, SC], F32R, tag="xt")
                        nc.gpsimd.dma_start(
                            out=xt_t, in_=xt_r[:, :, sc * SC:(sc + 1) * SC]
                        )
                        if b == 0 and sc == 0:
                            # stream the rest of the weights behind x chunk 0
                            nc.sync.dma_start(out=wk_t, in_=wk_r)
                            nc.sync.dma_start(out=wv_t, in_=wv_r)
                        for h in range(HL):
                            psq = psp.tile([128, SC], F32, tag="pq", bufs=2)
                            psk = psp.tile([128, SC], F32, tag="pk", bufs=2)
                            for dc in range(NDC):
                                _mm(nc, psq, wq_t[:, dc, h * 128:(h + 1) * 128],
                                    xt_t[:, dc, :], dc == 0, dc == NDC - 1)
                            for dc in range(NDC):
                                _mm(nc, psk, wk_t[:, dc, h * 128:(h + 1) * 128],
                                    xt_t[:, dc, :], dc == 0, dc == NDC - 1)
                            nc.scalar.activation(
                                out=qt[:, h, sc * SC:(sc + 1) * SC], in_=psq,
                                func=AF.Identity, bias=bqt[:, h:h + 1], scale=1.0)
                            nc.vector.tensor_copy(
                                kt[:, h, sc * SC:(sc + 1) * SC], psk)
                        for sb in range(SC // 128):
                            kb = sc * (SC // 128) + sb
                            psv = psp.tile([128, DHL], F32, tag="pv", bufs=2)
                            for dc in range(NDC):
                                _mm(nc, psv, xt_t[:, dc, sb * 128:(sb + 1) * 128],
                                    wv_t[:, dc, :], dc == 0, dc == NDC - 1)
                            nc.vector.tensor_copy(
                                vcat[:, kb, 0:128], psv[:, 0:128])
                            nc.vector.tensor_copy(
                                vcat[:, kb, 129:257], psv[:, 128:256])

                if b == 0:
                    nc.sync.dma_start(out=wo_t[:, 0, :], in_=wo[0])
                    nc.sync.dma_start(out=wo_t[:, 1, :], in_=wo[1])
                    nc.sync.dma_start(out=mask, in_=mask_d)
                    nc.sync.dma_start(out=ident, in_=ident_d)

                if dbg:
                    nc.sync.dma_start(out=qt_dump[b], in_=qt.bitcast(F32))
                    nc.sync.dma_start(out=kt_dump[b], in_=kt.bitcast(F32))
                    nc.sync.dma_start(out=vcat_dump[b], in_=vcat.bitcast(F32))

                # ---- causal attention + inline output projection ----
                with tc.tile_pool(name=f"ex{b}", bufs=3) as xpool, \
                     tc.tile_pool(name=f"sm{b}", bufs=2) as sm, \
                     tc.tile_pool(name=f"psa{b}", bufs=1, space="PSUM") as psa:
                    for qi in range(NQI):
                        st = {}
                        for h in range(HL):
                            vlo = 0 if h == 0 else 1
                            a0 = 0 if h == 0 else 128   # attn cols in acc
                            dn = 128 if h == 0 else 127  # denom col in acc
                            accs = [psa.tile([128, 256], F32, tag=f"acc{i}",
                                             bufs=1, name=f"acc{i}")
                                    for i in range(4)]
                            for kb in range(4 * qi + 4):
                                dq = max(0, (kb - 4 * qi)) * 128
                                pss = psa.tile([128, 512], F32, tag="sc", bufs=2)
                                _mm(nc, pss, kt[:, h, kb * 128:(kb + 1) * 128],
                                    qt[:, h, qi * 512:(qi + 1) * 512], True, True)
                                ex = xpool.tile([128, 512], F32R, tag="ex", bufs=6)
                                for half in range(2):
                                    lo = max(dq, half * 256)
                                    hi = (half + 1) * 256
                                    if lo >= hi:
                                        continue
                                    nc.scalar.activation(
                                        out=ex[:, lo:hi], in_=pss[:, lo:hi],
                                        func=AF.Exp)
                                if kb >= 4 * qi:
                                    nc.gpsimd.tensor_mul(
                                        ex[:, dq:dq + 128],
                                        ex[:, dq:dq + 128], mask)
                                for qql in range(4):
                                    qq = 4 * qi + qql
                                    if kb <= qq:
                                        _mm(nc, accs[qql],
                                            ex[:, qql * 128:(qql + 1) * 128],
                                            vcat[:, kb, vlo:vlo + 256],
                                            kb == 0, kb == qq)
                            for qql in range(4):
                                qq = 4 * qi + qql
                                rc = sm.tile([128, 1], F32, tag="rc")
                                nc.vector.reciprocal(rc, accs[qql][:, dn:dn + 1])
                                an = sm.tile([128, 128], F32R, tag="an")
                                nc.scalar.activation(
                                    out=an, in_=accs[qql][:, a0:a0 + 128],
                                    func=AF.Copy, scale=rc)
                                if dbg:
                                    nc.sync.dma_start(
                                        out=an_dump[b, h,
                                                    qq * 128:(qq + 1) * 128, :],
                                        in_=an.bitcast(F32))
                                    nc.sync.dma_start(out=rc_dump[b, h, qq],
                                                      in_=rc)
                                pst = psa.tile([128, 128], F32R, tag="sc", bufs=2)
                                nc.tensor.transpose(pst, an, ident)
                                stt = sm.tile([128, 128], F32R, tag="st", bufs=10)
                                nc.vector.tensor_copy(stt, pst)
                                st[(h, qql)] = stt
                        # output projection for these 4 q-blocks
                        for qql in range(4):
                            qq = 4 * qi + qql
                            for dk in range(D // 512):
                                po = psa.tile([128, 512], F32, tag="po", bufs=2)
                                _mm(nc, po, st[(0, qql)],
                                    wo_t[:, 0, dk * 512:(dk + 1) * 512],
                                    True, False)
                                _mm(nc, po, st[(1, qql)],
                                    wo_t[:, 1, dk * 512:(dk + 1) * 512],
                                    False, True)
                                ot = sm.tile([128, 512], F32, tag="ot", bufs=4)
                                nc.vector.tensor_copy(ot, po)
                                nc.sync.dma_start(
                                    out=out[b, qq * 128:(qq + 1) * 128,
                                            dk * 512:(dk + 1) * 512],
                                    in_=ot)

    nc.compile()
    return nc


def _get_nc():
    global _cached_nc
    if _cached_nc is None:
        _cached_nc = build_nc()
    return _cached_nc


def make_in_maps(X, Wq, bq, Wk, bk, Wv, bv, Wo, bo):
    X = np.ascontiguousarray(np.asarray(X, dtype=np.float32))
    scale = np.float32(1.0 / np.sqrt(DH))
    XT = np.ascontiguousarray(X.transpose(0, 2, 1))
    mask = (np.arange(128)[None, :] >= np.arange(128)[:, None]).astype(np.float32)
    ident = np.eye(128, dtype=np.float32)
    in_maps = []
    for c in range(N_CORES):
        hs = slice(c * DHL, (c + 1) * DHL)
        in_maps.append({
            "xt": XT,
            "wq": np.ascontiguousarray(np.asarray(Wq, np.float32)[:, hs] * scale),
            "wk": np.ascontiguousarray(np.asarray(Wk, np.float32)[:, hs]),
            "wv": np.ascontiguousarray(np.asarray(Wv, np.float32)[:, hs]),
            "bqt": np.ascontiguousarray(
                (np.asarray(bq, np.float32)[hs] * scale).reshape(HL, 128).T),
            "wo": np.ascontiguousarray(
                np.asarray(Wo, np.float32)[hs, :].reshape(HL, 128, D)),
            "maskt": mask,
            "ident": ident,
            "vones": np.ones((128, 1, 1), np.float32),
        })
    return in_maps


def kernel(X, Wq, bq, Wk, bk, Wv, bv, Wo, bo, _trace=False):
    nc = _get_nc()
    in_maps = make_in_maps(X, Wq, bq, Wk, bk, Wv, bv, Wo, bo)
    res = run_bass_kernel_spmd(nc, in_maps, list(range(N_CORES)), trace=_trace)
    acc = res.results[0]["out"].astype(np.float64)
    for c in range(1, N_CORES):
        acc += res.results[c]["out"]
    # bv commutes through softmax: sum_k w_k (v_k + bv) = (sum_k w_k v_k) + bv,
    # so the V bias contributes bv @ Wo, folded here with bo.
    acc += np.asarray(bo, np.float64) + (
        np.asarray(bv, np.float64) @ np.asarray(Wo, np.float64))
    out = acc.astype(np.float32)
    if _trace:
        return out, res
    return out



# revision 3
# speedup vs baseline: 1.3612x; 1.3612x over previous
"""Trainium2 Bass kernel for multi-head causal self-attention.

Tensor-parallel over 8 NeuronCores: each core owns 2 of the 16 heads.
Per core (SPMD, identical program, different weight shards), fused
chunk pipeline per batch: for each 512-row s-chunk, QKV projections
for that chunk, then causal attention for that q-chunk over all key
blocks produced so far, with the output projection inlined.

All matmuls run in bf16 (fp32 PSUM accumulation): same 1-column/cycle
stream rate as fp32r but LDWEIGHTS gets FWL (2x), DMA bytes halve.
Scores kept transposed [k, q]; softmax denominator via a ones column
appended to each head's V block ([V_h | 1], 129-wide PV matmuls).
Host: shards weights, pre-transposes X to bf16, sums 8 bf16 partials,
folds in bo + bv @ Wo (bv commutes through softmax; bk cancels).
"""
import numpy as np
from contextlib import ExitStack

import concourse.bass as bass
import concourse.tile as tile
from concourse import bacc, mybir
from concourse.bass_utils import run_bass_kernel_spmd

# Problem shape (hardcoded per contract)
B, S, D = 2, 2048, 2048
H, DH = 16, 128
N_CORES = 8
HL = H // N_CORES          # heads per core = 2
DHL = HL * DH              # 256
SC = 512                   # s-chunk width
NSC = S // SC              # 4 chunks per batch
NKB = S // 128             # 16 key blocks per batch
NDC = D // 128             # 16 contraction blocks

F32 = mybir.dt.float32
BF16 = mybir.dt.bfloat16
AF = mybir.ActivationFunctionType

_cached_nc = None


def _mm(nc, out, lhsT, rhs, start, stop):
    nc.tensor.matmul(out, lhsT, rhs, start=start, stop=stop)


def build_nc():
    nc = bacc.Bacc("TRN2", target_bir_lowering=False, debug=False,
                   num_devices=N_CORES)

    xt = nc.dram_tensor("xt", [B, D, S], BF16, kind="ExternalInput").ap()
    wq = nc.dram_tensor("wq", [D, DHL], BF16, kind="ExternalInput").ap()
    wk = nc.dram_tensor("wk", [D, DHL], BF16, kind="ExternalInput").ap()
    wv = nc.dram_tensor("wv", [D, DHL], BF16, kind="ExternalInput").ap()
    bqt_d = nc.dram_tensor("bqt", [128, HL], F32, kind="ExternalInput").ap()
    wo = nc.dram_tensor("wo", [HL, 128, D], BF16, kind="ExternalInput").ap()
    mask_d = nc.dram_tensor("maskt", [128, 128], BF16, kind="ExternalInput").ap()
    ident_d = nc.dram_tensor("ident", [128, 128], BF16, kind="ExternalInput").ap()
    out = nc.dram_tensor("out", [B, S, D], BF16, kind="ExternalOutput").ap()

    wq_r = wq.rearrange("(n p) d -> p n d", p=128)
    wk_r = wk.rearrange("(n p) d -> p n d", p=128)
    wv_r = wv.rearrange("(n p) d -> p n d", p=128)

    with tile.TileContext(nc) as tc, ExitStack() as ctx:
        pp = ctx.enter_context(tc.tile_pool(name="persist", bufs=1))

        wq_t = pp.tile([128, NDC, DHL], BF16)
        wk_t = pp.tile([128, NDC, DHL], BF16)
        wv_t = pp.tile([128, NDC, DHL], BF16)
        wo_t = pp.tile([128, HL, D], BF16)
        bqt = pp.tile([128, HL], F32)
        mask = pp.tile([128, 128], BF16)
        ident = pp.tile([128, 128], BF16)

        # critical-path-first DMA order: wq gates the very first matmul
        nc.sync.dma_start(out=wq_t[:, 0:4, :], in_=wq_r[:, 0:4, :])
        nc.sync.dma_start(out=wq_t[:, 4:NDC, :], in_=wq_r[:, 4:NDC, :])
        nc.sync.dma_start(out=bqt, in_=bqt_d)
        nc.sync.dma_start(out=wk_t, in_=wk_r)
        nc.sync.dma_start(out=wv_t, in_=wv_r)
        nc.sync.dma_start(out=mask, in_=mask_d)
        nc.sync.dma_start(out=ident, in_=ident_d)
        nc.sync.dma_start(out=wo_t[:, 0, :], in_=wo[0])
        nc.sync.dma_start(out=wo_t[:, 1, :], in_=wo[1])

        # double-buffered across batches
        qt_b = [pp.tile([128, HL, S], BF16, name=f"qt{i}") for i in range(2)]
        kt_b = [pp.tile([128, HL, S], BF16, name=f"kt{i}") for i in range(2)]
        vcat_b = [pp.tile([128, NKB, HL, 129], BF16, name=f"vc{i}")
                  for i in range(2)]

        xp = ctx.enter_context(tc.tile_pool(name="xtp", bufs=3))
        qkvp = ctx.enter_context(
            tc.tile_pool(name="qkvp", bufs=2, space="PSUM"))
        scp = ctx.enter_context(tc.tile_pool(name="scp", bufs=2, space="PSUM"))
        accp = ctx.enter_context(tc.tile_pool(name="accp", bufs=1, space="PSUM"))
        exp_ = ctx.enter_context(tc.tile_pool(name="exp", bufs=6))
        sm = ctx.enter_context(tc.tile_pool(name="sm", bufs=2))

        for b in range(B):
            qt, kt, vcat = qt_b[b % 2], kt_b[b % 2], vcat_b[b % 2]
            nc.gpsimd.memset(vcat[:, :, :, 128:129], 1.0)

            xt_r = xt[b].rearrange("(n p) s -> p n s", p=128)

            for sc in range(NSC):
                # ---- QKV projections for this chunk ----
                xt_t = xp.tile([128, NDC, SC], BF16, tag="xt")
                for part in range(4):
                    nc.gpsimd.dma_start(
                        out=xt_t[:, 4 * part:4 * part + 4, :],
                        in_=xt_r[:, 4 * part:4 * part + 4,
                                 sc * SC:(sc + 1) * SC])
                for h in range(HL):
                    psq = qkvp.tile([128, SC], F32, tag="qkv")
                    for dc in range(NDC):
                        _mm(nc, psq, wq_t[:, dc, h * 128:(h + 1) * 128],
                            xt_t[:, dc, :], dc == 0, dc == NDC - 1)
                    nc.scalar.activation(
                        out=qt[:, h, sc * SC:(sc + 1) * SC], in_=psq,
                        func=AF.Identity, bias=bqt[:, h:h + 1], scale=1.0)
                    psk = qkvp.tile([128, SC], F32, tag="qkv")
                    for dc in range(NDC):
                        _mm(nc, psk, wk_t[:, dc, h * 128:(h + 1) * 128],
                            xt_t[:, dc, :], dc == 0, dc == NDC - 1)
                    nc.vector.tensor_copy(
                        kt[:, h, sc * SC:(sc + 1) * SC], psk)
                for sb in range(SC // 128):
                    kb = sc * (SC // 128) + sb
                    psv = qkvp.tile([128, DHL], F32, tag="qkv")
                    for dc in range(NDC):
                        _mm(nc, psv, xt_t[:, dc, sb * 128:(sb + 1) * 128],
                            wv_t[:, dc, :], dc == 0, dc == NDC - 1)
                    nc.vector.tensor_copy(vcat[:, kb, 0, 0:128], psv[:, 0:128])
                    nc.vector.tensor_copy(vcat[:, kb, 1, 0:128],
                                          psv[:, 128:256])

                # ---- causal attention for q-chunk qi = sc ----
                qi = sc
                st = {}
                for h in range(HL):
                    accs = [accp.tile([128, 129], F32, tag=f"acc{i}",
                                      name=f"acc{i}")
                            for i in range(4)]
                    for kb in range(4 * qi + 4):
                        dq = max(0, (kb - 4 * qi)) * 128
                        pss = scp.tile([128, SC], F32, tag="sc")
                        _mm(nc, pss[:, dq:SC], kt[:, h, kb * 128:(kb + 1) * 128],
                            qt[:, h, qi * SC + dq:(qi + 1) * SC], True, True)
                        ex = exp_.tile([128, SC], BF16, tag="ex")
                        if dq == 0:
                            for half in range(2):
                                nc.scalar.activation(
                                    out=ex[:, half * 256:(half + 1) * 256],
                                    in_=pss[:, half * 256:(half + 1) * 256],
                                    func=AF.Exp)
                        else:
                            nc.scalar.activation(
                                out=ex[:, dq:SC], in_=pss[:, dq:SC],
                                func=AF.Exp)
                        if kb >= 4 * qi:
                            nc.gpsimd.tensor_mul(
                                ex[:, dq:dq + 128], ex[:, dq:dq + 128], mask)
                        for qql in range(4):
                            qq = 4 * qi + qql
                            if kb <= qq:
                                _mm(nc, accs[qql],
                                    ex[:, qql * 128:(qql + 1) * 128],
                                    vcat[:, kb, h, :],
                                    kb == 0, kb == qq)
                    for qql in range(4):
                        rc = sm.tile([128, 1], F32, tag="rc", bufs=4)
                        nc.vector.reciprocal(rc, accs[qql][:, 128:129])
                        an_s = sm.tile([128, 128], BF16, tag="an", bufs=4)
                        nc.vector.tensor_scalar_mul(
                            an_s, accs[qql][:, 0:128], rc)
                        pst = scp.tile([128, 128], BF16, tag="sc")
                        nc.tensor.transpose(pst, an_s, ident)
                        stt = sm.tile([128, 128], BF16, tag="st", bufs=10)
                        nc.vector.tensor_copy(stt, pst)
                        st[(h, qql)] = stt
                # ---- output projection for these 4 q-blocks ----
                for qql in range(4):
                    qq = 4 * qi + qql
                    for dk in range(D // 512):
                        po = scp.tile([128, 512], F32, tag="sc")
                        _mm(nc, po, st[(0, qql)],
                            wo_t[:, 0, dk * 512:(dk + 1) * 512], True, False)
                        _mm(nc, po, st[(1, qql)],
                            wo_t[:, 1, dk * 512:(dk + 1) * 512], False, True)
                        ot = sm.tile([128, 512], BF16, tag="ot", bufs=4)
                        if dk % 2 == 0:
                            nc.vector.tensor_copy(ot, po)
                        else:
                            nc.scalar.activation(out=ot, in_=po, func=AF.Copy)
                        nc.sync.dma_start(
                            out=out[b, qq * 128:(qq + 1) * 128,
                                    dk * 512:(dk + 1) * 512],
                            in_=ot)

    nc.compile()
    return nc


def _get_nc():
    global _cached_nc
    if _cached_nc is None:
        _cached_nc = build_nc()
    return _cached_nc


def make_in_maps(X, Wq, bq, Wk, bk, Wv, bv, Wo, bo):
    import ml_dtypes
    bf16 = ml_dtypes.bfloat16
    X = np.asarray(X, dtype=np.float32)
    scale = np.float32(1.0 / np.sqrt(DH))
    XT = np.ascontiguousarray(X.transpose(0, 2, 1)).astype(bf16)
    mask = (np.arange(128)[None, :] >= np.arange(128)[:, None]).astype(bf16)
    ident = np.eye(128, dtype=bf16)
    in_maps = []
    for c in range(N_CORES):
        hs = slice(c * DHL, (c + 1) * DHL)
        in_maps.append({
            "xt": XT,
            "wq": np.ascontiguousarray(
                np.asarray(Wq, np.float32)[:, hs] * scale).astype(bf16),
            "wk": np.ascontiguousarray(
                np.asarray(Wk, np.float32)[:, hs]).astype(bf16),
            "wv": np.ascontiguousarray(
                np.asarray(Wv, np.float32)[:, hs]).astype(bf16),
            "bqt": np.ascontiguousarray(
                (np.asarray(bq, np.float32)[hs] * scale).reshape(HL, 128).T),
            "wo": np.ascontiguousarray(
                np.asarray(Wo, np.float32)[hs, :].reshape(HL, 128, D)
            ).astype(bf16),
            "maskt": mask,
            "ident": ident,
        })
    return in_maps


def kernel(X, Wq, bq, Wk, bk, Wv, bv, Wo, bo, _trace=False):
    nc = _get_nc()
    in_maps = make_in_maps(X, Wq, bq, Wk, bk, Wv, bv, Wo, bo)
    res = run_bass_kernel_spmd(nc, in_maps, list(range(N_CORES)), trace=_trace)
    acc = res.results[0]["out"].astype(np.float64)
    for c in range(1, N_CORES):
        acc += res.results[c]["out"].astype(np.float64)
    # bv commutes through softmax: sum_k w_k (v_k + bv) = (sum_k w_k v_k) + bv,
    # so the V bias contributes bv @ Wo, folded here with bo.
    acc += np.asarray(bo, np.float64) + (
        np.asarray(bv, np.float64) @ np.asarray(Wo, np.float64))
    out = acc.astype(np.float32)
    if _trace:
        return out, res
    return out
